# revision 30
# baseline (speedup 1.0000x reference)
"""Trainium2 Bass kernel for DigitConvolutionalModel.

Math: logits = relu(conv2d_valid(x.reshape(B,28,28), conv_w).reshape(B,676) @ W1 + b1) @ W2 + b2

Optimizations:
  1. The valid 3x3 conv is linear in x, so it folds into W1 on host:
     feat @ W1 == x @ (C @ W1) where C[784,676] scatters conv_w taps.
     The device then runs two dense matmuls per batch shard:
       h = relu(x @ W1eff + b1);  logits = h @ W2  (b2 added on host)
  2. Sharding layout: batch 32768 split as 8 x 4096 across cores; each
     shard is fed to its core pre-transposed and pre-tiled so the
     contraction dim lands on SBUF partitions with no on-device
     transposes of x.
  3. Contraction 784 = 6*128 + 16: six full-K chunks plus a 16-row
     tail. The two m-halves' tails run as row-tiles at array rows
     0-31 / 32-63 back to back (concurrent in the PE), so MM1 costs
     ~13 N=512 passes per block instead of 14 at K=112 x 7. Tail
     pixels for all blocks live in one compact [48, 4096] tensor
     (16 rows + a copy at rows 32:48) - no per-block zero padding.
  4. x is float8e3 (e3m4: 4 mantissa bits, range +-15.5 fits N(0,1))
     on device; weights/h stay bf16 (mixed-dtype matmul is legal when
     neither operand is fp32). This halves x DMA bytes vs bf16 and,
     measured, keeps end-to-end rel err at ~1.44e-2 (host-sim exact
     match with HW). bf16 x was 3.6e-3 but ~6us slower: the front of
     the kernel is DMA-bandwidth-bound, so x bytes are wall-clock.
  5. DMA schedule: the first matmuls' operands head both rings
     (block0's tail slice + w1t), then block0's chunks and w1 stream
     in halves; bulk blocks split across the sync and scalar rings.
     Everything is issued up front; the queues run saturated from
     ~8us until the stream is done.
  6. A few warmup matmuls on scratch SBUF bridge kernel entry until
     block0 lands, so the PE HAM clock-gate (cold = 1.2GHz) starts
     its ~3.4us un-throttle window as early as possible.
  7. MM2 for block n runs at the START of block n+2 (before the tail
     pair), so (a) its wait on block n's relu is long satisfied and
     (b) the weight-stream discontinuities of MM2 and the tail merge
     at one point of the block instead of two - fewer LDWEIGHTS
     pipeline stalls (a weight swap that can't prefetch costs ~100ns).
  8. logits leave PSUM via a vector-engine copy (DVE is otherwise
     idle) and DMA out per block on the sync ring (idle after the x
     stream; relu on scalar never queues behind descriptor gen).
     b2 is added on host during the final transpose. The final block
     splits MM2 into column halves whose drains alternate rings.

Device kernel (per core, per 512-column block):
  - MM2 of block n-2: logitsT[10,512] over 2 K=128 chunks -> PSUM,
    DVE copy to SBUF, DMA out
  - MM1: K=16 tail pair (row-tiles) + 6 K=128 chunks per m-half
  - ACT: relu(hT + b1) PSUM->SBUF per m-half, output bf16
"""
import ml_dtypes
import numpy as np

import concourse.bacc as bacc
import concourse.mybir as mybir
from concourse.tile import TileContext
from concourse.bass_utils import run_bass_kernel_spmd

B = 32768
IMG = 28
KSZ = 3
OUT_HW = IMG - KSZ + 1  # 26
FEAT = OUT_HW * OUT_HW  # 676
PIX = IMG * IMG  # 784
HID = 256
NCLS = 10
N_CORES = 8
BC = B // N_CORES  # 4096 rows per core
NBLK_COLS = 512  # batch columns per pipeline block (1 PSUM bank of fp32)
KCH = 128  # full-partition contraction chunks
NKC = 6  # six full chunks cover pixels 0..767
KTAIL = PIX - NKC * KCH  # 16 tail pixels

f32 = mybir.dt.float32
bf16 = mybir.dt.bfloat16
AF = mybir.ActivationFunctionType

X_DT = mybir.dt.float8e3  # e3m4: 4 mantissa bits, range +-15.5 fits N(0,1) x
W_DT = bf16
H_DT = bf16

N_WARM = 4  # warmup matmuls bridging kernel entry -> block0 readiness
MM2_LAG = 2  # blocks between MM1 and its MM2 (relu wait amortization)

_CACHE = {}


def _build(bc=BC):
    """Build the single-core Bass program (SPMD across 8 cores)."""
    nblk = bc // NBLK_COLS
    nc = bacc.Bacc()
    # x blob: [nblk, 128, 6, 512] - block-major, partition p of chunk c
    # holds pixel c*128+p; one contiguous 384KB DMA per block.
    xT = nc.declare_dram_parameter("xT", [nblk, KCH, NKC, NBLK_COLS], X_DT,
                                   isOutput=False)
    # tail blob [48, bc]: pixels 768:784 for every column, rows 0:16,
    # duplicated at rows 32:48 (for the second row-tile), rest unused.
    xTt = nc.declare_dram_parameter("xTt", [48, bc], X_DT, isOutput=False)
    # w1 blob: [128, 6, 256] (chunk-major per partition)
    w1e = nc.declare_dram_parameter("w1b", [KCH, NKC, HID], W_DT, isOutput=False)
    # w1 tail blob [48, 128]: rows 0:16 = W1eff[768:784, 0:128],
    # rows 32:48 = W1eff[768:784, 128:256]
    w1t = nc.declare_dram_parameter("w1t", [48, KCH], W_DT, isOutput=False)
    # w2 blob: [128, 2, 10] (chunk-major per partition)
    w2 = nc.declare_dram_parameter("w2b", [128, 2, NCLS], W_DT, isOutput=False)
    # b1 blob: [128, 2] fp32 (column mc holds b1[mc*128:(mc+1)*128])
    wb = nc.declare_dram_parameter("wb", [128, 2], f32, isOutput=False)
    # output is logitsT [10, bc] WITHOUT b2; host adds b2 + transposes
    out = nc.declare_dram_parameter("out", [NCLS, bc], f32, isOutput=True)

    with TileContext(nc) as tc:
        with (
            tc.tile_pool(name="weights", bufs=1) as wpool,
            tc.tile_pool(name="xt_sb", bufs=8) as xtpool,
            tc.tile_pool(name="h_sb", bufs=6) as hpool,
            tc.tile_pool(name="h_ps", bufs=4, space="PSUM") as hps,
            tc.tile_pool(name="log_ps", bufs=3, space="PSUM") as logps,
            tc.tile_pool(name="log_sb", bufs=3) as logsb,
        ):
            w1_sb = wpool.tile([KCH, NKC, HID], W_DT)
            w1t_sb = wpool.tile([48, KCH], W_DT)
            w2_sb = wpool.tile([128, 2, NCLS], W_DT)
            wb_sb = wpool.tile([128, 2], f32)
            xtt_sb = wpool.tile([48, bc], X_DT)
            xts = []
            for _ in range(nblk):
                xt_blk = xtpool.tile([KCH, NKC, NBLK_COLS], X_DT, tag="xt")
                xts.append(xt_blk)
            xt0 = xts[0]
            # ---- DMA schedule. The first matmuls' operands (block0's
            # tail slice, w1t) head both rings; block0 + w1 follow in
            # halves; bulk blocks split across the rings. ----
            nc.sync.dma_start(out=xtt_sb[:, 0:NBLK_COLS],
                              in_=xTt[:, 0:NBLK_COLS])
            nc.scalar.dma_start(out=w1t_sb[:], in_=w1t[:])
            # block0 streams per-chunk (fine arrival granularity keeps the
            # PE gap-free while the HAM un-throttle window runs), paired
            # with the matching w1 chunk on the scalar ring
            for kc in range(NKC):
                nc.sync.dma_start(out=xt0[:, kc, :], in_=xT[0, :, kc, :])
                nc.scalar.dma_start(out=w1_sb[:, kc, :], in_=w1e[:, kc, :])
            nc.scalar.dma_start(out=wb_sb[:], in_=wb[:])
            nc.scalar.dma_start(out=w2_sb[:], in_=w2[:])
            # rest of the tails, block1 in halves, bulk blocks whole
            nc.sync.dma_start(out=xtt_sb[:, NBLK_COLS:], in_=xTt[:, NBLK_COLS:])
            nc.sync.dma_start(out=xts[1][:, 0:3, :], in_=xT[1, :, 0:3, :])
            nc.sync.dma_start(out=xts[1][:, 3:6, :], in_=xT[1, :, 3:6, :])
            for blk in range(2, nblk):
                nc.sync.dma_start(out=xts[blk][:], in_=xT[blk])
            b1_sb = wb_sb

            # HAM warm-up: dummy matmuls on scratch data bridge the short
            # window until block0's tail slice lands. Memset on gpsimd -
            # it clears the tile-context entry barrier earliest and is
            # otherwise idle.
            warm = wpool.tile([128, NBLK_COLS], W_DT)
            nc.gpsimd.memset(warm[:], 0.0)
            warm_ps = hps.tile([128, NBLK_COLS], f32, tag="h_ps")
            for _ in range(N_WARM):
                nc.tensor.matmul(
                    warm_ps[:], warm[:, 0:128], warm[:], start=True, stop=True,
                    skip_group_check=True,
                )

            # ---- main pipeline over 512-column blocks ----
            pending = []  # [(hs, b0), ...] awaiting MM2, oldest first

            def emit_mm2_batch(items, split=False, last=False):
                # Batch MM2 for several blocks: chunk-major MM order
                # (c0 of every item, then c1 of every item) so each w2
                # chunk's LDWEIGHTS is paid once per batch. split=True
                # (final block): column halves pipeline the PSUM drain
                # behind the second half's matmuls.
                parts = []  # (log_ps, hs, b0, c0, w)
                for hs, b0 in items:
                    halves = (
                        [(0, NBLK_COLS // 2), (NBLK_COLS // 2, NBLK_COLS // 2)]
                        if split
                        else [(0, NBLK_COLS)]
                    )
                    for c0, w in halves:
                        log_ps = logps.tile([NCLS, NBLK_COLS], f32,
                                            tag="log_ps")
                        parts.append((log_ps, hs, b0, c0, w))
                for mc in range(2):
                    for log_ps, hs, b0, c0, w in parts:
                        nc.tensor.matmul(
                            log_ps[:, 0:w],
                            w2_sb[:, mc, :],
                            hs[mc][:, c0 : c0 + w],
                            start=(mc == 0),
                            stop=(mc == 1),
                            skip_group_check=True,
                        )
                for hi, (log_ps, hs, b0, c0, w) in enumerate(parts):
                    # PSUM -> SBUF on the otherwise-idle vector engine
                    log_sb = logsb.tile([NCLS, NBLK_COLS], f32, tag="log_sb")
                    nc.vector.tensor_copy(out=log_sb[:, 0:w], in_=log_ps[:, 0:w])
                    # drains ride the sync ring (idle after the x blocks)
                    # so relu on scalar never queues behind descriptor gen
                    eng = nc.scalar if (last and hi == len(parts) - 1) else nc.sync
                    eng.dma_start(
                        out=out[:, b0 + c0 : b0 + c0 + w],
                        in_=log_sb[:, 0:w],
                    )

            for blk in range(nblk):
                b0 = blk * NBLK_COLS
                xt = xts[blk]
                # MM2s for a PAIR of ready blocks lead every other block:
                # one weight-stream discontinuity (and one w2 LDW pair)
                # per two blocks, merged with the tail pair's. Pairs fire
                # at blocks 3,5,7 (lag >= 2 for both members), leaving
                # only the last two blocks to flush after the loop.
                if blk % 2 == 1 and len(pending) >= MM2_LAG + 1:
                    emit_mm2_batch([pending.pop(0), pending.pop(0)])

                h_ps0 = hps.tile([128, NBLK_COLS], f32, tag="h_ps")
                h_ps1 = hps.tile([128, NBLK_COLS], f32, tag="h_ps")
                h_ps = [h_ps0, h_ps1]
                # K=16 tail pair: row-tiles at array rows 0-31 / 32-63,
                # adjacent in program order -> concurrent in the PE.
                nc.tensor.matmul(
                    h_ps[0][:], w1t_sb[0:KTAIL, :],
                    xtt_sb[0:KTAIL, b0 : b0 + NBLK_COLS],
                    start=True, stop=False, skip_group_check=True,
                )
                nc.tensor.matmul(
                    h_ps[1][:], w1t_sb[32 : 32 + KTAIL, :],
                    xtt_sb[32 : 32 + KTAIL, b0 : b0 + NBLK_COLS],
                    start=True, stop=False, skip_group_check=True,
                )
                hs = []
                for mc in range(2):
                    for kc in range(NKC):
                        nc.tensor.matmul(
                            h_ps[mc][:],
                            w1_sb[:, kc, mc * 128 : (mc + 1) * 128],
                            xt[:, kc, :],
                            start=False,
                            stop=(kc == NKC - 1),
                            skip_group_check=True,
                        )
                    h_sb = hpool.tile([128, NBLK_COLS], H_DT, tag="h")
                    nc.scalar.activation(
                        h_sb[:], h_ps[mc][:], AF.Relu,
                        bias=b1_sb[:, mc : mc + 1],
                    )
                    hs.append(h_sb)
                pending.append((hs, b0))

            while len(pending) > 1:
                emit_mm2_batch([pending.pop(0)])
            emit_mm2_batch([pending.pop(0)], split=True, last=True)

    nc.compile()
    return nc


def _fold_conv_into_w1(conv_w, W1):
    """W1eff[784, 256] such that x @ W1eff == conv(x) flattened @ W1."""
    conv_w = np.asarray(conv_w, dtype=np.float64)
    W1 = np.asarray(W1, dtype=np.float64)
    C = np.zeros((IMG, IMG, OUT_HW, OUT_HW), dtype=np.float64)
    oi = np.arange(OUT_HW)[:, None]
    oj = np.arange(OUT_HW)[None, :]
    for ki in range(KSZ):
        for kj in range(KSZ):
            C[oi + ki, oj + kj, oi, oj] = conv_w[ki, kj]
    W1eff = C.reshape(PIX, FEAT) @ W1
    return np.ascontiguousarray(W1eff, dtype=np.float32)


def _pack_weights(w1e, b1, W2):
    np_wdt = mybir.dt.np(W_DT)
    # w1 blob [128, 6, 256]: chunk-major per partition (pixels 0..767)
    w1b = np.ascontiguousarray(
        w1e[: NKC * KCH].reshape(NKC, KCH, HID).transpose(1, 0, 2).astype(np_wdt)
    )
    # w1 tail blob [48, 128]
    w1t = np.zeros((48, KCH), dtype=np_wdt)
    w1t[0:KTAIL] = w1e[NKC * KCH :, 0:128].astype(np_wdt)
    w1t[32 : 32 + KTAIL] = w1e[NKC * KCH :, 128:256].astype(np_wdt)
    w2b = np.ascontiguousarray(
        W2.reshape(2, 128, NCLS).transpose(1, 0, 2).astype(np_wdt)
    )
    wb = np.ascontiguousarray(b1.reshape(2, 128).T.astype(np.float32))
    return w1b, w1t, w2b, wb


def _pack_x(xc, nblk):
    """Per-core shard [bc, 784] -> ([nblk,128,6,512] blob, [48,bc] tail)."""
    np_xdt = mybir.dt.np(X_DT)
    xs = xc.reshape(nblk, NBLK_COLS, PIX)
    main = np.ascontiguousarray(
        xs[:, :, : NKC * KCH]
        .reshape(nblk, NBLK_COLS, NKC, KCH)
        .transpose(0, 3, 2, 1)
        .astype(np_xdt)
    )
    tail = np.zeros((48, nblk * NBLK_COLS), dtype=np_xdt)
    t = (
        xs[:, :, NKC * KCH :]
        .transpose(2, 0, 1)
        .reshape(KTAIL, nblk * NBLK_COLS)
        .astype(np_xdt)
    )
    tail[0:KTAIL] = t
    tail[32 : 32 + KTAIL] = t
    return main, tail


def kernel(x, conv_w, W1, b1, W2, b2, _bc=BC, _trace=False):
    x = np.asarray(x, dtype=np.float32)
    w1e = _fold_conv_into_w1(conv_w, W1)
    b1 = np.asarray(b1, dtype=np.float32)
    W2 = np.asarray(W2, dtype=np.float32)
    b2 = np.asarray(b2, dtype=np.float32)
    w1b, w1t, w2b, wb = _pack_weights(w1e, b1, W2)

    n_cores = x.shape[0] // _bc
    if _bc not in _CACHE:
        _CACHE[_bc] = _build(_bc)
    nc = _CACHE[_bc]

    nblk = _bc // NBLK_COLS
    in_maps = []
    for c in range(n_cores):
        main, tail = _pack_x(x[c * _bc : (c + 1) * _bc], nblk)
        in_maps.append(
            {"xT": main, "xTt": tail, "w1b": w1b, "w1t": w1t,
             "w2b": w2b, "wb": wb}
        )
    res = run_bass_kernel_spmd(
        nc, in_maps, core_ids=list(range(n_cores)), trace=_trace
    )
    # device layout logitsT [10, bc] (no b2) -> [bc, 10] + b2
    out = np.concatenate(
        [np.ascontiguousarray(res.results[c]["out"].T) for c in range(n_cores)],
        axis=0,
    ) + b2[None, :]
    if _trace:
        return out, res
    return out


# revision 33
# speedup vs baseline: 1.1575x; 1.1575x over previous
"""Trainium2 Bass kernel for DigitConvolutionalModel.

Math: logits = relu(conv2d_valid(x.reshape(B,28,28), conv_w).reshape(B,676) @ W1 + b1) @ W2 + b2

Optimizations:
  1. The valid 3x3 conv is linear in x, so it folds into W1 on host:
     feat @ W1 == x @ (C @ W1) where C[784,676] scatters conv_w taps.
     The device then runs two dense matmuls per batch shard:
       h = relu(x @ W1eff + b1);  logits = h @ W2  (b2 added on host)
  2. Sharding layout: batch 32768 split as 8 x 4096 across cores; each
     shard is fed to its core pre-transposed and pre-tiled so the
     contraction dim lands on SBUF partitions with no on-device
     transposes of x.
  3. Contraction 784 = 6*128 + 16: six full-K chunks plus a 16-row
     tail. The two m-halves' tails run as row-tiles at array rows
     0-31 / 32-63 back to back (concurrent in the PE), so MM1 costs
     ~13 N=512 passes per block instead of 14 at K=112 x 7. Tail
     pixels for all blocks live in one compact [48, 4096] tensor
     (16 rows + a copy at rows 32:48) - no per-block zero padding.
  4. x is float8e3 (e3m4: 4 mantissa bits, range +-15.5 fits N(0,1))
     on device; weights/h stay bf16 (mixed-dtype matmul is legal when
     neither operand is fp32). This halves x DMA bytes vs bf16 and,
     measured, keeps end-to-end rel err at ~1.44e-2 (host-sim exact
     match with HW). bf16 x was 3.6e-3 but ~6us slower: the front of
     the kernel is DMA-bandwidth-bound, so x bytes are wall-clock.
  5. DMA schedule: the first matmuls' operands head both rings
     (block0's tail slice + w1t), then block0's chunks and w1 stream
     in halves; bulk blocks split across the sync and scalar rings.
     Everything is issued up front; the queues run saturated from
     ~8us until the stream is done.
  6. A few warmup matmuls on scratch SBUF bridge kernel entry until
     block0 lands, so the PE HAM clock-gate (cold = 1.2GHz) starts
     its ~3.4us un-throttle window as early as possible.
  7. MM2 for block n runs at the START of block n+2 (before the tail
     pair), so (a) its wait on block n's relu is long satisfied and
     (b) the weight-stream discontinuities of MM2 and the tail merge
     at one point of the block instead of two - fewer LDWEIGHTS
     pipeline stalls (a weight swap that can't prefetch costs ~100ns).
  8. logits leave PSUM via a vector-engine copy (DVE is otherwise
     idle) and DMA out per block on the sync ring (idle after the x
     stream; relu on scalar never queues behind descriptor gen).
     b2 is added on host during the final transpose. The final block
     splits MM2 into column halves whose drains alternate rings.

Device kernel (per core, per 512-column block):
  - MM2 of block n-2: logitsT[10,512] over 2 K=128 chunks -> PSUM,
    DVE copy to SBUF, DMA out
  - MM1: K=16 tail pair (row-tiles) + 6 K=128 chunks per m-half
  - ACT: relu(hT + b1) PSUM->SBUF per m-half, output bf16
"""
import ml_dtypes
import numpy as np

import concourse.bacc as bacc
import concourse.mybir as mybir
from concourse.tile import TileContext
from concourse.bass_utils import run_bass_kernel_spmd

B = 32768
IMG = 28
KSZ = 3
OUT_HW = IMG - KSZ + 1  # 26
FEAT = OUT_HW * OUT_HW  # 676
PIX = IMG * IMG  # 784
HID = 256
NCLS = 10
N_CORES = 8
BC = B // N_CORES  # 4096 rows per core
NBLK_COLS = 512  # batch columns per pipeline block (1 PSUM bank of fp32)
KCH = 128  # full-partition contraction chunks
NKC = 6  # six full chunks cover pixels 0..767
KTAIL = PIX - NKC * KCH  # 16 tail pixels

f32 = mybir.dt.float32
bf16 = mybir.dt.bfloat16
AF = mybir.ActivationFunctionType

X_DT = mybir.dt.float8e3  # e3m4: 4 mantissa bits, range +-15.5 fits N(0,1) x
W_DT = bf16
H_DT = bf16

N_WARM = 4  # warmup matmuls bridging kernel entry -> block0 readiness
MM2_LAG = 2  # blocks between MM1 and its MM2 (relu wait amortization)

_CACHE = {}


def _build(bc=BC):
    """Build the single-core Bass program (SPMD across 8 cores)."""
    nblk = bc // NBLK_COLS
    nc = bacc.Bacc()
    # x blob: [nblk, 128, 6, 512] - block-major, partition p of chunk c
    # holds pixel c*128+p; one contiguous 384KB DMA per block.
    xT = nc.declare_dram_parameter("xT", [nblk, KCH, NKC, NBLK_COLS], X_DT,
                                   isOutput=False)
    # tail blob [48, bc]: pixels 768:784 for every column, rows 0:16,
    # duplicated at rows 32:48 (for the second row-tile), rest unused.
    xTt = nc.declare_dram_parameter("xTt", [48, bc], X_DT, isOutput=False)
    # w1 blob: [128, 6, 256] (chunk-major per partition)
    w1e = nc.declare_dram_parameter("w1b", [KCH, NKC, HID], W_DT, isOutput=False)
    # w1 tail blob [48, 128]: rows 0:16 = W1eff[768:784, 0:128],
    # rows 32:48 = W1eff[768:784, 128:256]
    w1t = nc.declare_dram_parameter("w1t", [48, KCH], W_DT, isOutput=False)
    # w2 blob: [128, 2, 10] (chunk-major per partition)
    w2 = nc.declare_dram_parameter("w2b", [128, 2, NCLS], W_DT, isOutput=False)
    # b1 blob: [128, 2] fp32 (column mc holds b1[mc*128:(mc+1)*128])
    wb = nc.declare_dram_parameter("wb", [128, 2], f32, isOutput=False)
    # output is logitsT [10, bc] WITHOUT b2; host adds b2 + transposes
    out = nc.declare_dram_parameter("out", [NCLS, bc], f32, isOutput=True)

    with TileContext(nc) as tc:
        with (
            tc.tile_pool(name="weights", bufs=1) as wpool,
            tc.tile_pool(name="xt_sb", bufs=8) as xtpool,
            tc.tile_pool(name="h_sb", bufs=6) as hpool,
            tc.tile_pool(name="h_ps", bufs=4, space="PSUM") as hps,
            tc.tile_pool(name="log_ps", bufs=3, space="PSUM") as logps,
            tc.tile_pool(name="log_sb", bufs=3) as logsb,
        ):
            w1_sb = wpool.tile([KCH, NKC, HID], W_DT)
            w1t_sb = wpool.tile([48, KCH], W_DT)
            w2_sb = wpool.tile([128, 2, NCLS], W_DT)
            wb_sb = wpool.tile([128, 2], f32)
            xtt_sb = wpool.tile([48, bc], X_DT)
            xts = []
            for _ in range(nblk):
                xt_blk = xtpool.tile([KCH, NKC, NBLK_COLS], X_DT, tag="xt")
                xts.append(xt_blk)
            xt0 = xts[0]
            # ---- DMA schedule. The first matmuls' operands (block0's
            # tail slice, w1t) head both rings; block0 + w1 follow in
            # halves; bulk blocks split across the rings. ----
            nc.sync.dma_start(out=xtt_sb[:, 0:NBLK_COLS],
                              in_=xTt[:, 0:NBLK_COLS])
            nc.scalar.dma_start(out=w1t_sb[:], in_=w1t[:])
            # block0 streams per-chunk (fine arrival granularity keeps the
            # PE gap-free while the HAM un-throttle window runs), paired
            # with the matching w1 chunk on the scalar ring
            for kc in range(NKC):
                nc.sync.dma_start(out=xt0[:, kc, :], in_=xT[0, :, kc, :])
                nc.scalar.dma_start(out=w1_sb[:, kc, :], in_=w1e[:, kc, :])
            nc.scalar.dma_start(out=wb_sb[:], in_=wb[:])
            nc.scalar.dma_start(out=w2_sb[:], in_=w2[:])
            # block1's tail slice + halves, then the remaining tails and
            # bulk blocks whole
            nc.sync.dma_start(out=xtt_sb[:, NBLK_COLS : 2 * NBLK_COLS],
                              in_=xTt[:, NBLK_COLS : 2 * NBLK_COLS])
            nc.sync.dma_start(out=xts[1][:, 0:3, :], in_=xT[1, :, 0:3, :])
            nc.sync.dma_start(out=xts[1][:, 3:6, :], in_=xT[1, :, 3:6, :])
            nc.sync.dma_start(out=xtt_sb[:, 2 * NBLK_COLS :],
                              in_=xTt[:, 2 * NBLK_COLS :])
            for blk in range(2, nblk):
                nc.sync.dma_start(out=xts[blk][:], in_=xT[blk])
            b1_sb = wb_sb

            # HAM warm-up: dummy matmuls on scratch data bridge the short
            # window until block0's tail slice lands. Memset on gpsimd -
            # it clears the tile-context entry barrier earliest and is
            # otherwise idle.
            warm = wpool.tile([128, NBLK_COLS], W_DT)
            nc.gpsimd.memset(warm[:], 0.0)
            warm_ps = hps.tile([128, NBLK_COLS], f32, tag="h_ps")
            for _ in range(N_WARM):
                nc.tensor.matmul(
                    warm_ps[:], warm[:, 0:128], warm[:], start=True, stop=True,
                    skip_group_check=True,
                )

            # ---- main pipeline over 512-column blocks ----
            pending = []  # [(hs, b0), ...] awaiting MM2, oldest first

            def emit_mm2_batch(items, split=False, last=False):
                # Batch MM2 for several blocks: chunk-major MM order
                # (c0 of every item, then c1 of every item) so each w2
                # chunk's LDWEIGHTS is paid once per batch. split=True
                # (final block): column halves pipeline the PSUM drain
                # behind the second half's matmuls.
                parts = []  # (log_ps, hs, b0, c0, w)
                for hs, b0 in items:
                    halves = (
                        [(0, NBLK_COLS // 2), (NBLK_COLS // 2, NBLK_COLS // 2)]
                        if split
                        else [(0, NBLK_COLS)]
                    )
                    for c0, w in halves:
                        log_ps = logps.tile([NCLS, NBLK_COLS], f32,
                                            tag="log_ps")
                        parts.append((log_ps, hs, b0, c0, w))
                for mc in range(2):
                    for log_ps, hs, b0, c0, w in parts:
                        nc.tensor.matmul(
                            log_ps[:, 0:w],
                            w2_sb[:, mc, :],
                            hs[mc][:, c0 : c0 + w],
                            start=(mc == 0),
                            stop=(mc == 1),
                            skip_group_check=True,
                        )
                for hi, (log_ps, hs, b0, c0, w) in enumerate(parts):
                    # PSUM -> SBUF on the otherwise-idle vector engine
                    log_sb = logsb.tile([NCLS, NBLK_COLS], f32, tag="log_sb")
                    nc.vector.tensor_copy(out=log_sb[:, 0:w], in_=log_ps[:, 0:w])
                    # drains ride the sync ring (idle after the x blocks)
                    # so relu on scalar never queues behind descriptor gen
                    eng = nc.scalar if (last and hi == len(parts) - 1) else nc.sync
                    eng.dma_start(
                        out=out[:, b0 + c0 : b0 + c0 + w],
                        in_=log_sb[:, 0:w],
                    )

            for blk in range(nblk):
                b0 = blk * NBLK_COLS
                xt = xts[blk]
                # MM2 of block n-2 leads the block: its weight-stream
                # discontinuity merges with the tail pair's.
                if len(pending) >= MM2_LAG:
                    emit_mm2_batch([pending.pop(0)])

                h_ps0 = hps.tile([128, NBLK_COLS], f32, tag="h_ps")
                h_ps1 = hps.tile([128, NBLK_COLS], f32, tag="h_ps")
                h_ps = [h_ps0, h_ps1]
                # K=16 tail pair: row-tiles at array rows 0-31 / 32-63,
                # adjacent in program order -> concurrent in the PE.
                nc.tensor.matmul(
                    h_ps[0][:], w1t_sb[0:KTAIL, :],
                    xtt_sb[0:KTAIL, b0 : b0 + NBLK_COLS],
                    start=True, stop=False, skip_group_check=True,
                )
                nc.tensor.matmul(
                    h_ps[1][:], w1t_sb[32 : 32 + KTAIL, :],
                    xtt_sb[32 : 32 + KTAIL, b0 : b0 + NBLK_COLS],
                    start=True, stop=False, skip_group_check=True,
                )
                hs = []
                for mc in range(2):
                    for kc in range(NKC):
                        nc.tensor.matmul(
                            h_ps[mc][:],
                            w1_sb[:, kc, mc * 128 : (mc + 1) * 128],
                            xt[:, kc, :],
                            start=False,
                            stop=(kc == NKC - 1),
                            skip_group_check=True,
                        )
                    h_sb = hpool.tile([128, NBLK_COLS], H_DT, tag="h")
                    nc.scalar.activation(
                        h_sb[:], h_ps[mc][:], AF.Relu,
                        bias=b1_sb[:, mc : mc + 1],
                    )
                    hs.append(h_sb)
                pending.append((hs, b0))

            while len(pending) > 1:
                emit_mm2_batch([pending.pop(0)])
            emit_mm2_batch([pending.pop(0)], split=True, last=True)

    nc.compile()
    return nc


def _fold_conv_into_w1(conv_w, W1):
    """W1eff[784, 256] such that x @ W1eff == conv(x) flattened @ W1."""
    conv_w = np.asarray(conv_w, dtype=np.float64)
    W1 = np.asarray(W1, dtype=np.float64)
    C = np.zeros((IMG, IMG, OUT_HW, OUT_HW), dtype=np.float64)
    oi = np.arange(OUT_HW)[:, None]
    oj = np.arange(OUT_HW)[None, :]
    for ki in range(KSZ):
        for kj in range(KSZ):
            C[oi + ki, oj + kj, oi, oj] = conv_w[ki, kj]
    W1eff = C.reshape(PIX, FEAT) @ W1
    return np.ascontiguousarray(W1eff, dtype=np.float32)


def _pack_weights(w1e, b1, W2):
    np_wdt = mybir.dt.np(W_DT)
    # w1 blob [128, 6, 256]: chunk-major per partition (pixels 0..767)
    w1b = np.ascontiguousarray(
        w1e[: NKC * KCH].reshape(NKC, KCH, HID).transpose(1, 0, 2).astype(np_wdt)
    )
    # w1 tail blob [48, 128]
    w1t = np.zeros((48, KCH), dtype=np_wdt)
    w1t[0:KTAIL] = w1e[NKC * KCH :, 0:128].astype(np_wdt)
    w1t[32 : 32 + KTAIL] = w1e[NKC * KCH :, 128:256].astype(np_wdt)
    w2b = np.ascontiguousarray(
        W2.reshape(2, 128, NCLS).transpose(1, 0, 2).astype(np_wdt)
    )
    wb = np.ascontiguousarray(b1.reshape(2, 128).T.astype(np.float32))
    return w1b, w1t, w2b, wb


def _pack_x(xc, nblk):
    """Per-core shard [bc, 784] -> ([nblk,128,6,512] blob, [48,bc] tail)."""
    np_xdt = mybir.dt.np(X_DT)
    xs = xc.reshape(nblk, NBLK_COLS, PIX)
    main = np.ascontiguousarray(
        xs[:, :, : NKC * KCH]
        .reshape(nblk, NBLK_COLS, NKC, KCH)
        .transpose(0, 3, 2, 1)
        .astype(np_xdt)
    )
    tail = np.zeros((48, nblk * NBLK_COLS), dtype=np_xdt)
    t = (
        xs[:, :, NKC * KCH :]
        .transpose(2, 0, 1)
        .reshape(KTAIL, nblk * NBLK_COLS)
        .astype(np_xdt)
    )
    tail[0:KTAIL] = t
    tail[32 : 32 + KTAIL] = t
    return main, tail


def kernel(x, conv_w, W1, b1, W2, b2, _bc=BC, _trace=False):
    x = np.asarray(x, dtype=np.float32)
    w1e = _fold_conv_into_w1(conv_w, W1)
    b1 = np.asarray(b1, dtype=np.float32)
    W2 = np.asarray(W2, dtype=np.float32)
    b2 = np.asarray(b2, dtype=np.float32)
    w1b, w1t, w2b, wb = _pack_weights(w1e, b1, W2)

    n_cores = x.shape[0] // _bc
    if _bc not in _CACHE:
        _CACHE[_bc] = _build(_bc)
    nc = _CACHE[_bc]

    nblk = _bc // NBLK_COLS
    in_maps = []
    for c in range(n_cores):
        main, tail = _pack_x(x[c * _bc : (c + 1) * _bc], nblk)
        in_maps.append(
            {"xT": main, "xTt": tail, "w1b": w1b, "w1t": w1t,
             "w2b": w2b, "wb": wb}
        )
    res = run_bass_kernel_spmd(
        nc, in_maps, core_ids=list(range(n_cores)), trace=_trace
    )
    # device layout logitsT [10, bc] (no b2) -> [bc, 10] + b2
    out = np.concatenate(
        [np.ascontiguousarray(res.results[c]["out"].T) for c in range(n_cores)],
        axis=0,
    ) + b2[None, :]
    if _trace:
        return out, res
    return out
